# revision 29
# baseline (speedup 1.0000x reference)
"""Trainium2 Bass kernel: low-rank (LoRA-style) linear with 2:4 soft-threshold
pruned weights, fp16 matmul / fp32 accumulate.

  wA = soft_threshold24(weight_A) * scale_A          # [IN, R]
  wB = soft_threshold24(weight_B) * scale_B          # [OUT, R]
  x_proj = f16(x) @ f16(wA)            (f32 accum)   # [N, R]
  out    = f16(x_proj) @ f16(wB).T + bias            # [N, OUT]

Sharding: data-parallel over the token dim across 8 cores (2048 tokens/core),
small weights replicated. No collectives.

v4 key idea: weights are DMAed CONTIGUOUSLY in "p-major" layout (partition p
holds rows 32p..32p+31; 128 fat descriptors instead of 4096 x 256B strided
ones).  Instead of re-blocking the weights on chip, the COMPUTE adapts:
 - mm1 contracts features in p-major order: the x transposes pick stride-32
   column views (columns {32p+c}), so each xT slab lines up with a p-major
   wa16pm slice directly.  Strided column addressing is free for the PE.
 - mm2 streams wB.T from a c-major wbtF (contiguous ACT copies off the PE
   transposes) through a strided rhs access pattern that enumerates columns
   in real feature order, so PSUM/ob/out-DMA all stay natural.
x input: SWDGE cast-DMA f32(HBM)->f16(SBUF) per 2MB tile; f16 single-pass PE
transposes; PSUM->SBUF copies split DVE/ACT; 1MB half-tile out DMAs (sync).
"""

import sys

import numpy as np

if "/opt/trn_rl_repo" not in sys.path:
    sys.path.insert(0, "/opt/trn_rl_repo")

B, S, IN_F, OUT_F, RANK = 4, 4096, 4096, 4096, 64
N_CORES = 8
N_TOK = B * S                   # 16384
T_CORE = N_TOK // N_CORES       # 2048 tokens per core
P = 128
TT = 2                          # token tiles per group
GTOK = TT * P                   # 256 tokens per group
N_GRP = T_CORE // GTOK          # 8 groups per core
N_IB = IN_F // P                # 32 stride-32 column families (c values)
MM2_N = 512
N_OB = OUT_F // MM2_N           # 8 output column groups

_CACHE = {}


def _soft_threshold_f16(nc, ve, pool, stage, scale, out_f16):
    """out_f16 = soft_threshold24(w) * scale, elementwise in f16 (stage is
    already f16 via cast-DMA; 2x DVE throughput vs f32, error ~1e-3 << tol).
    Layout [P, 32, RANK] -- elementwise, so any row blocking works."""
    import concourse.mybir as mybir

    f16 = mybir.dt.bfloat16
    nb = stage.shape[1]
    amin = mybir.AluOpType.min
    amx = mybir.AluOpType.max

    wfh = stage[:]
    wneg = pool.tile([P, nb, RANK], f16, tag="wneg", name="wneg")
    ve.tensor_scalar_mul(wneg[:], wfh, -1.0)
    aw = pool.tile([P, nb, RANK], f16, tag="awabs", name="awabs")
    ve.tensor_tensor(aw[:], wfh, wneg[:], op=amx)
    a4 = aw[:].rearrange("p b (g q) -> p b g q", q=4)
    ab = [a4[:, :, :, j : j + 1] for j in range(4)]
    ash = [P, nb, RANK // 4, 1]
    m1 = pool.tile(ash, f16, tag="m1", name="m1")
    M1 = pool.tile(ash, f16, tag="M1", name="M1")
    m2 = pool.tile(ash, f16, tag="m2", name="m2")
    M2 = pool.tile(ash, f16, tag="M2", name="M2")
    ve.tensor_tensor(m1[:], ab[0], ab[1], op=amin)
    ve.tensor_tensor(M1[:], ab[0], ab[1], op=amx)
    ve.tensor_tensor(m2[:], ab[2], ab[3], op=amin)
    ve.tensor_tensor(M2[:], ab[2], ab[3], op=amx)
    # 2nd smallest of the 4 = min(max(m1, m2), min(M1, M2))
    t = pool.tile(ash, f16, tag="tq", name="t")
    ve.tensor_tensor(m1[:], m1[:], m2[:], op=amx)
    ve.tensor_tensor(M1[:], M1[:], M2[:], op=amin)
    ve.tensor_tensor(t[:], m1[:], M1[:], op=amin)
    # t4: threshold broadcast over the group-of-4 axis
    t4 = pool.tile([P, nb, RANK], f16, tag="t4", name="t4")
    h4 = t4[:].rearrange("p b (g q) -> p b g q", q=4)
    for j in range(4):
        ve.tensor_copy(h4[:, :, :, j : j + 1], t[:])
    # s = w - clip(w, -t, t);  -t4 reuses wneg's slot
    nt4 = pool.tile([P, nb, RANK], f16, tag="wneg", name="nt4")
    ve.tensor_scalar_mul(nt4[:], t4[:], -1.0)
    thr = pool.tile([P, nb, RANK], f16, tag="awabs", name="wthr")
    th = thr[:]
    ve.tensor_tensor(th, wfh, t4[:], op=amin)
    ve.tensor_tensor(th, th, nt4[:], op=amx)
    if scale != 1.0:
        ve.tensor_sub(th, wfh, th)
        ve.tensor_scalar_mul(out_f16[:], th, float(scale))
    else:
        ve.tensor_sub(out_f16[:], wfh, th)


def _build(scale_a, scale_b):
    import concourse.mybir as mybir
    import concourse.tile as tile
    from concourse import bacc
    from concourse.bass import ts
    from concourse.masks import make_identity

    f32, f16 = mybir.dt.float32, mybir.dt.bfloat16

    nc = bacc.Bacc("TRN2", target_bir_lowering=False, debug=False,
                   enable_asserts=False)
    x_d = nc.dram_tensor("x", [T_CORE, IN_F], f32, kind="ExternalInput")
    wa_d = nc.dram_tensor("weight_A", [IN_F, RANK], f32, kind="ExternalInput")
    wb_d = nc.dram_tensor("weight_B", [OUT_F, RANK], f32, kind="ExternalInput")
    b_d = nc.dram_tensor("bias", [1, OUT_F], f32, kind="ExternalInput")
    o_d = nc.dram_tensor("out", [T_CORE, OUT_F], f32, kind="ExternalOutput")

    with tile.TileContext(nc) as tc:
        with (
            tc.tile_pool(name="const", bufs=1) as constp,
            tc.tile_pool(name="wtmp", bufs=1) as wtmp,
            tc.tile_pool(name="xin", bufs=5) as xinp,
            tc.tile_pool(name="xtp", bufs=3) as xtp,
            tc.tile_pool(name="outp", bufs=4) as outp,
            tc.tile_pool(name="proj", bufs=6) as projp,
            tc.tile_pool(name="pst", bufs=3, space="PSUM") as pst,
            tc.tile_pool(name="ps1", bufs=2, space="PSUM") as ps1p,
            tc.tile_pool(name="ps2", bufs=3, space="PSUM") as ps2p,
        ):
            # --- weight stage loads interleaved ahead of the x stream on
            # the gpsimd FIFO: block-major (256B descriptors), cast f32->f16
            # during DMA.  FIFO priority makes them immune to the round-robin
            # starvation behind fat x packets that parallel queues suffer.
            x2s = []

            def load_x2(g):
                """One 4MB cast-DMA delivering both 128-token tiles of
                group g (fatter transfer, fewer per-DMA overheads)."""
                x2 = xinp.tile([P, TT, IN_F], f16, name="x2")
                nc.gpsimd.dma_start(
                    x2[:],
                    x_d[ts(g, TT * P), :].rearrange("(pair p) f -> p pair f",
                                                    p=P))
                x2s.append(x2)

            load_x2(0)
            wstg_a = wtmp.tile([P, N_IB, RANK], f16, tag="wstg_a", name="wsa")
            nc.gpsimd.dma_start(wstg_a[:],
                               wa_d[:].rearrange("(b p) r -> p b r", p=P))
            wstg_b = wtmp.tile([P, N_IB, RANK], f16, tag="wstg_b", name="wsb")
            nc.gpsimd.dma_start(wstg_b[:],
                               wb_d[:].rearrange("(b p) r -> p b r", p=P))

            ident16 = constp.tile([P, P], f16)
            make_identity(nc, ident16[:])

            # wbt: wB.T in natural column order (+ bias row filled by a
            # contiguous cast-DMA f32->f16 on the gpsimd queue).
            wbt = constp.tile([RANK + 1, OUT_F], f16)
            nc.gpsimd.dma_start(wbt[RANK : RANK + 1, :], b_d[:])

            for g in range(1, N_GRP):
                load_x2(g)

            # --- weight preprocessing on DVE (elementwise, block-major) ---
            wa16 = wtmp.tile([P, N_IB, RANK], f16, tag="wa16", name="wa16")
            _soft_threshold_f16(nc, nc.vector, wtmp, wstg_a, scale_a, wa16)
            wb16 = wtmp.tile([P, N_IB, RANK], f16, tag="wb16", name="wb16")
            _soft_threshold_f16(nc, nc.vector, wtmp, wstg_b, scale_b, wb16)

            xTs = {}

            def emit_transposes(g):
                """PE bf16 transposes of group g; full-bank PSUM tiles (8
                transposes per tile) drained by ACT/DVE alternating.  While
                DVE still runs the weight chains (first two groups), ACT
                takes every copy so the transpose pipe never waits on DVE."""
                xT = xtp.tile([P, N_IB, GTOK], f16, name="xT")
                for q in range(N_IB // 8):
                    for tt in range(TT):
                        pt = pst.tile([P, 8 * P], f16, tag="ptx", name="pt")
                        for bb in range(8):
                            b = 8 * q + bb
                            nc.tensor.transpose(
                                pt[:, ts(bb, P)],
                                x2s[g][:, tt, ts(b, P)], ident16[:])
                        dst = xT[:, 8 * q : 8 * q + 8, ts(tt, P)]
                        src = pt[:].rearrange("p (a b) -> p a b", a=8)
                        if g < 2 or (q + tt) % 2 == 0:
                            nc.scalar.copy(dst, src)
                        else:
                            nc.vector.tensor_copy(out=dst, in_=src)
                xTs[g] = xT

            xpas = {}

            def emit_mm1(g):
                ps1 = ps1p.tile([RANK, GTOK], f32)
                for b in range(N_IB):
                    nc.tensor.matmul(ps1[:], wa16[:, b, :], xTs[g][:, b, :],
                                     start=(b == 0), stop=(b == N_IB - 1))
                xpa = projp.tile([RANK + 1, GTOK], f16)
                nc.vector.tensor_copy(out=xpa[0:RANK, :], in_=ps1[:])
                nc.vector.memset(xpa[RANK : RANK + 1, :], 1.0)
                xpas[g] = xpa

            def emit_wbt():
                """PE transposes of block-major wb16 into natural-order wbt
                rows 0..63 -- ACT copies are fully contiguous."""
                for q in range(N_IB // 8):
                    pw = pst.tile([P, 8 * P], f16, tag="ptx", name="pw")
                    for bb in range(8):
                        b = 8 * q + bb
                        nc.tensor.transpose(pw[:RANK, ts(bb, P)],
                                            wb16[:, b, :], ident16[:])
                    nc.scalar.copy(wbt[0:RANK, ts(q, 8 * P)],
                                   pw[:RANK, :])

            def emit_mm2(g, drain_chunks=False):
                for tt in range(TT):
                    i = g * TT + tt
                    for h in range(2):
                        ob = outp.tile([P, OUT_F // 2], f32, name="ob",
                                       tag="ob")
                        for jj in range(N_OB // 2):
                            j = h * (N_OB // 2) + jj
                            ps2 = ps2p.tile([P, MM2_N], f32, tag="ps2",
                                            name="ps2")
                            nc.tensor.matmul(ps2[:], xpas[g][:, ts(tt, P)],
                                             wbt[:, ts(j, MM2_N)],
                                             start=True, stop=True)
                            if j % 2 == 0:
                                nc.vector.tensor_copy(
                                    out=ob[:, ts(jj, MM2_N)], in_=ps2[:])
                            else:
                                nc.scalar.copy(ob[:, ts(jj, MM2_N)], ps2[:])
                            if drain_chunks:
                                nc.sync.dma_start(
                                    o_d[ts(i, P), ts(j, MM2_N)],
                                    ob[:, ts(jj, MM2_N)])
                        if not drain_chunks:
                            nc.sync.dma_start(
                                o_d[ts(i, P),
                                    h * (OUT_F // 2) : (h + 1) * (OUT_F // 2)],
                                ob[:])

            # --- static schedule: transposes run ahead while weights
            # preprocess; mm1 follows wa16pm, mm2 follows wbtF.
            emit_transposes(0)
            emit_transposes(1)
            emit_mm1(0)
            emit_transposes(2)
            emit_mm1(1)
            emit_wbt()
            emit_mm2(0)
            emit_transposes(3)
            emit_mm1(2)
            emit_transposes(4)
            emit_mm1(3)
            emit_mm2(1)
            emit_transposes(5)
            emit_mm1(4)
            emit_mm2(2)
            emit_transposes(6)
            emit_mm1(5)
            emit_mm2(3)
            emit_transposes(7)
            emit_mm1(6)
            emit_mm2(4)
            emit_mm1(7)
            emit_mm2(5)
            emit_mm2(6)
            emit_mm2(7, drain_chunks=True)

    nc.compile()
    return nc


def get_nc(scale_a, scale_b):
    key = (float(scale_a), float(scale_b))
    if key not in _CACHE:
        _CACHE[key] = _build(*key)
    return _CACHE[key]


def kernel(x, weight_A, weight_B, bias, scale_A, scale_B):
    from concourse.bass_utils import run_bass_kernel_spmd

    x = np.ascontiguousarray(np.asarray(x, dtype=np.float32))
    wa = np.ascontiguousarray(np.asarray(weight_A, dtype=np.float32))
    wb = np.ascontiguousarray(np.asarray(weight_B, dtype=np.float32))
    bi = np.ascontiguousarray(np.asarray(bias, dtype=np.float32)).reshape(1, OUT_F)
    sa = float(np.asarray(scale_A))
    sb = float(np.asarray(scale_B))

    nc = get_nc(sa, sb)

    xf = x.reshape(N_TOK, IN_F)
    in_maps = [
        {
            "x": xf[c * T_CORE : (c + 1) * T_CORE],
            "weight_A": wa,
            "weight_B": wb,
            "bias": bi,
        }
        for c in range(N_CORES)
    ]
    res = run_bass_kernel_spmd(nc, in_maps, core_ids=list(range(N_CORES)))
    out = np.concatenate([r["out"] for r in res.results], axis=0)
    return out.reshape(B, S, OUT_F)


# revision 30
# speedup vs baseline: 1.1029x; 1.1029x over previous
"""Trainium2 Bass kernel: low-rank (LoRA-style) linear with 2:4 soft-threshold
pruned weights, fp16 matmul / fp32 accumulate.

  wA = soft_threshold24(weight_A) * scale_A          # [IN, R]
  wB = soft_threshold24(weight_B) * scale_B          # [OUT, R]
  x_proj = f16(x) @ f16(wA)            (f32 accum)   # [N, R]
  out    = f16(x_proj) @ f16(wB).T + bias            # [N, OUT]

Sharding: data-parallel over the token dim across 8 cores (2048 tokens/core),
small weights replicated. No collectives.

v4 key idea: weights are DMAed CONTIGUOUSLY in "p-major" layout (partition p
holds rows 32p..32p+31; 128 fat descriptors instead of 4096 x 256B strided
ones).  Instead of re-blocking the weights on chip, the COMPUTE adapts:
 - mm1 contracts features in p-major order: the x transposes pick stride-32
   column views (columns {32p+c}), so each xT slab lines up with a p-major
   wa16pm slice directly.  Strided column addressing is free for the PE.
 - mm2 streams wB.T from a c-major wbtF (contiguous ACT copies off the PE
   transposes) through a strided rhs access pattern that enumerates columns
   in real feature order, so PSUM/ob/out-DMA all stay natural.
x input: SWDGE cast-DMA f32(HBM)->f16(SBUF) per 2MB tile; f16 single-pass PE
transposes; PSUM->SBUF copies split DVE/ACT; 1MB half-tile out DMAs (sync).
"""

import sys

import numpy as np

if "/opt/trn_rl_repo" not in sys.path:
    sys.path.insert(0, "/opt/trn_rl_repo")

B, S, IN_F, OUT_F, RANK = 4, 4096, 4096, 4096, 64
N_CORES = 8
N_TOK = B * S                   # 16384
T_CORE = N_TOK // N_CORES       # 2048 tokens per core
P = 128
TT = 2                          # token tiles per group
GTOK = TT * P                   # 256 tokens per group
N_GRP = T_CORE // GTOK          # 8 groups per core
N_IB = IN_F // P                # 32 stride-32 column families (c values)
MM2_N = 512
N_OB = OUT_F // MM2_N           # 8 output column groups

_CACHE = {}


def _soft_threshold_f16(nc, ve, pool, stage, scale, out_f16):
    """out_f16 = soft_threshold24(w) * scale, elementwise in f16 (stage is
    already f16 via cast-DMA; 2x DVE throughput vs f32, error ~1e-3 << tol).
    Layout [P, 32, RANK] -- elementwise, so any row blocking works."""
    import concourse.mybir as mybir

    f16 = mybir.dt.bfloat16
    nb = stage.shape[1]
    amin = mybir.AluOpType.min
    amx = mybir.AluOpType.max

    wfh = stage[:]
    wneg = pool.tile([P, nb, RANK], f16, tag="wneg", name="wneg")
    ve.tensor_scalar_mul(wneg[:], wfh, -1.0)
    aw = pool.tile([P, nb, RANK], f16, tag="awabs", name="awabs")
    ve.tensor_tensor(aw[:], wfh, wneg[:], op=amx)
    a4 = aw[:].rearrange("p b (g q) -> p b g q", q=4)
    ab = [a4[:, :, :, j : j + 1] for j in range(4)]
    ash = [P, nb, RANK // 4, 1]
    m1 = pool.tile(ash, f16, tag="m1", name="m1")
    M1 = pool.tile(ash, f16, tag="M1", name="M1")
    m2 = pool.tile(ash, f16, tag="m2", name="m2")
    M2 = pool.tile(ash, f16, tag="M2", name="M2")
    ve.tensor_tensor(m1[:], ab[0], ab[1], op=amin)
    ve.tensor_tensor(M1[:], ab[0], ab[1], op=amx)
    ve.tensor_tensor(m2[:], ab[2], ab[3], op=amin)
    ve.tensor_tensor(M2[:], ab[2], ab[3], op=amx)
    # 2nd smallest of the 4 = min(max(m1, m2), min(M1, M2))
    t = pool.tile(ash, f16, tag="tq", name="t")
    ve.tensor_tensor(m1[:], m1[:], m2[:], op=amx)
    ve.tensor_tensor(M1[:], M1[:], M2[:], op=amin)
    ve.tensor_tensor(t[:], m1[:], M1[:], op=amin)
    # t4: threshold broadcast over the group-of-4 axis
    t4 = pool.tile([P, nb, RANK], f16, tag="t4", name="t4")
    h4 = t4[:].rearrange("p b (g q) -> p b g q", q=4)
    for j in range(4):
        ve.tensor_copy(h4[:, :, :, j : j + 1], t[:])
    # s = w - clip(w, -t, t);  -t4 reuses wneg's slot
    nt4 = pool.tile([P, nb, RANK], f16, tag="wneg", name="nt4")
    ve.tensor_scalar_mul(nt4[:], t4[:], -1.0)
    thr = pool.tile([P, nb, RANK], f16, tag="awabs", name="wthr")
    th = thr[:]
    ve.tensor_tensor(th, wfh, t4[:], op=amin)
    ve.tensor_tensor(th, th, nt4[:], op=amx)
    if scale != 1.0:
        ve.tensor_sub(th, wfh, th)
        ve.tensor_scalar_mul(out_f16[:], th, float(scale))
    else:
        ve.tensor_sub(out_f16[:], wfh, th)


def _build(scale_a, scale_b):
    import concourse.mybir as mybir
    import concourse.tile as tile
    from concourse import bacc
    from concourse.bass import ts
    from concourse.masks import make_identity

    f32, f16 = mybir.dt.float32, mybir.dt.bfloat16

    nc = bacc.Bacc("TRN2", target_bir_lowering=False, debug=False,
                   enable_asserts=False)
    x_d = nc.dram_tensor("x", [T_CORE, IN_F], f32, kind="ExternalInput")
    wa_d = nc.dram_tensor("weight_A", [IN_F, RANK], f32, kind="ExternalInput")
    wb_d = nc.dram_tensor("weight_B", [OUT_F, RANK], f32, kind="ExternalInput")
    b_d = nc.dram_tensor("bias", [1, OUT_F], f32, kind="ExternalInput")
    o_d = nc.dram_tensor("out", [T_CORE, OUT_F], f32, kind="ExternalOutput")

    with tile.TileContext(nc) as tc:
        with (
            tc.tile_pool(name="const", bufs=1) as constp,
            tc.tile_pool(name="wtmp", bufs=1) as wtmp,
            tc.tile_pool(name="xin", bufs=5) as xinp,
            tc.tile_pool(name="xtp", bufs=3) as xtp,
            tc.tile_pool(name="outp", bufs=4) as outp,
            tc.tile_pool(name="proj", bufs=6) as projp,
            tc.tile_pool(name="pst", bufs=3, space="PSUM") as pst,
            tc.tile_pool(name="ps1", bufs=2, space="PSUM") as ps1p,
            tc.tile_pool(name="ps2", bufs=3, space="PSUM") as ps2p,
        ):
            # --- weight stage loads interleaved ahead of the x stream on
            # the gpsimd FIFO: block-major (256B descriptors), cast f32->f16
            # during DMA.  FIFO priority makes them immune to the round-robin
            # starvation behind fat x packets that parallel queues suffer.
            x2s = []

            def load_x2(g):
                """One 4MB cast-DMA delivering both 128-token tiles of
                group g (fatter transfer, fewer per-DMA overheads)."""
                x2 = xinp.tile([P, TT, IN_F], f16, name="x2")
                nc.gpsimd.dma_start(
                    x2[:],
                    x_d[ts(g, TT * P), :].rearrange("(pair p) f -> p pair f",
                                                    p=P))
                x2s.append(x2)

            wstg_a = wtmp.tile([P, N_IB, RANK], f16, tag="wstg_a", name="wsa")
            nc.gpsimd.dma_start(wstg_a[:],
                               wa_d[:].rearrange("(b p) r -> p b r", p=P))
            load_x2(0)
            wstg_b = wtmp.tile([P, N_IB, RANK], f16, tag="wstg_b", name="wsb")
            nc.gpsimd.dma_start(wstg_b[:],
                               wb_d[:].rearrange("(b p) r -> p b r", p=P))

            ident16 = constp.tile([P, P], f16)
            make_identity(nc, ident16[:])

            # wbt: wB.T in natural column order (+ bias row filled by a
            # contiguous cast-DMA f32->f16 on the gpsimd queue).
            wbt = constp.tile([RANK + 1, OUT_F], f16)
            nc.gpsimd.dma_start(wbt[RANK : RANK + 1, :], b_d[:])

            for g in range(1, N_GRP):
                load_x2(g)

            # --- weight preprocessing on DVE (elementwise, block-major) ---
            wa16 = wtmp.tile([P, N_IB, RANK], f16, tag="wa16", name="wa16")
            _soft_threshold_f16(nc, nc.vector, wtmp, wstg_a, scale_a, wa16)
            wb16 = wtmp.tile([P, N_IB, RANK], f16, tag="wb16", name="wb16")
            _soft_threshold_f16(nc, nc.vector, wtmp, wstg_b, scale_b, wb16)

            xTs = {}

            def emit_transposes(g):
                """PE bf16 transposes of group g; full-bank PSUM tiles (8
                transposes per tile) drained by ACT/DVE alternating.  While
                DVE still runs the weight chains (first two groups), ACT
                takes every copy so the transpose pipe never waits on DVE."""
                xT = xtp.tile([P, N_IB, GTOK], f16, name="xT")
                for q in range(N_IB // 8):
                    for tt in range(TT):
                        pt = pst.tile([P, 8 * P], f16, tag="ptx", name="pt")
                        for bb in range(8):
                            b = 8 * q + bb
                            nc.tensor.transpose(
                                pt[:, ts(bb, P)],
                                x2s[g][:, tt, ts(b, P)], ident16[:])
                        dst = xT[:, 8 * q : 8 * q + 8, ts(tt, P)]
                        src = pt[:].rearrange("p (a b) -> p a b", a=8)
                        if g < 2 or (q + tt) % 2 == 0:
                            nc.scalar.copy(dst, src)
                        else:
                            nc.vector.tensor_copy(out=dst, in_=src)
                xTs[g] = xT

            xpas = {}

            def emit_mm1(g):
                ps1 = ps1p.tile([RANK, GTOK], f32)
                for b in range(N_IB):
                    nc.tensor.matmul(ps1[:], wa16[:, b, :], xTs[g][:, b, :],
                                     start=(b == 0), stop=(b == N_IB - 1))
                xpa = projp.tile([RANK + 1, GTOK], f16)
                nc.vector.tensor_copy(out=xpa[0:RANK, :], in_=ps1[:])
                nc.vector.memset(xpa[RANK : RANK + 1, :], 1.0)
                xpas[g] = xpa

            def emit_wbt():
                """PE transposes of block-major wb16 into natural-order wbt
                rows 0..63 -- ACT copies are fully contiguous."""
                for q in range(N_IB // 8):
                    pw = pst.tile([P, 8 * P], f16, tag="ptx", name="pw")
                    for bb in range(8):
                        b = 8 * q + bb
                        nc.tensor.transpose(pw[:RANK, ts(bb, P)],
                                            wb16[:, b, :], ident16[:])
                    nc.scalar.copy(wbt[0:RANK, ts(q, 8 * P)],
                                   pw[:RANK, :])

            def emit_mm2(g, drain_chunks=False):
                for tt in range(TT):
                    i = g * TT + tt
                    for h in range(2):
                        ob = outp.tile([P, OUT_F // 2], f32, name="ob",
                                       tag="ob")
                        for jj in range(N_OB // 2):
                            j = h * (N_OB // 2) + jj
                            ps2 = ps2p.tile([P, MM2_N], f32, tag="ps2",
                                            name="ps2")
                            nc.tensor.matmul(ps2[:], xpas[g][:, ts(tt, P)],
                                             wbt[:, ts(j, MM2_N)],
                                             start=True, stop=True)
                            if j % 2 == 0:
                                nc.vector.tensor_copy(
                                    out=ob[:, ts(jj, MM2_N)], in_=ps2[:])
                            else:
                                nc.scalar.copy(ob[:, ts(jj, MM2_N)], ps2[:])
                            if drain_chunks:
                                nc.sync.dma_start(
                                    o_d[ts(i, P), ts(j, MM2_N)],
                                    ob[:, ts(jj, MM2_N)])
                        if not drain_chunks:
                            nc.sync.dma_start(
                                o_d[ts(i, P),
                                    h * (OUT_F // 2) : (h + 1) * (OUT_F // 2)],
                                ob[:])

            # --- static schedule: transposes run ahead while weights
            # preprocess; mm1 follows wa16pm, mm2 follows wbtF.
            emit_transposes(0)
            emit_transposes(1)
            emit_mm1(0)
            emit_transposes(2)
            emit_mm1(1)
            emit_wbt()
            emit_transposes(3)
            emit_mm1(2)
            emit_mm2(0)
            emit_transposes(4)
            emit_mm1(3)
            emit_mm2(1)
            emit_transposes(5)
            emit_mm1(4)
            emit_mm2(2)
            emit_transposes(6)
            emit_mm1(5)
            emit_mm2(3)
            emit_transposes(7)
            emit_mm1(6)
            emit_mm2(4)
            emit_mm1(7)
            emit_mm2(5)
            emit_mm2(6)
            emit_mm2(7, drain_chunks=True)

    nc.compile()
    return nc


def get_nc(scale_a, scale_b):
    key = (float(scale_a), float(scale_b))
    if key not in _CACHE:
        _CACHE[key] = _build(*key)
    return _CACHE[key]


def kernel(x, weight_A, weight_B, bias, scale_A, scale_B):
    from concourse.bass_utils import run_bass_kernel_spmd

    x = np.ascontiguousarray(np.asarray(x, dtype=np.float32))
    wa = np.ascontiguousarray(np.asarray(weight_A, dtype=np.float32))
    wb = np.ascontiguousarray(np.asarray(weight_B, dtype=np.float32))
    bi = np.ascontiguousarray(np.asarray(bias, dtype=np.float32)).reshape(1, OUT_F)
    sa = float(np.asarray(scale_A))
    sb = float(np.asarray(scale_B))

    nc = get_nc(sa, sb)

    xf = x.reshape(N_TOK, IN_F)
    in_maps = [
        {
            "x": xf[c * T_CORE : (c + 1) * T_CORE],
            "weight_A": wa,
            "weight_B": wb,
            "bias": bi,
        }
        for c in range(N_CORES)
    ]
    res = run_bass_kernel_spmd(nc, in_maps, core_ids=list(range(N_CORES)))
    out = np.concatenate([r["out"] for r in res.results], axis=0)
    return out.reshape(B, S, OUT_F)
